# revision 1
# baseline (speedup 1.0000x reference)
"""Batched LoRA Linear on 8 Trainium2 NeuronCores (Bass/Tile).

Computes, for x (32, 512, 4096), adapter_ids (32,), A_all (32, 16, 4096),
B_all (32, 4096, 16), W (4096, 4096), b (4096,):

    out = x @ W.T + b + 2.0 * ((x @ A[aid].T) @ B[aid].T)

Sharding: data-parallel over batch — 4 samples per core; W/b replicated.
Per-core device kernel (all fp32 data, fp32r matmuls, fp32 PSUM accum):
  XT (d_in on partitions) resident per 1024-token block; W^T streamed;
  LoRA path fused into the same PSUM accumulation before a single
  bias-add eviction.

Host side only reshapes/transposes/gathers (no arithmetic except the
exact *2.0 fold into B).
"""

import sys
import types

import numpy as np

# ---------------------------------------------------------------- constants
P = 128
B_SZ = 32            # batch
S = 512              # seq len
D_IN = 4096
D_OUT = 4096
RANK = 16
N_CORES = 8
SPB = B_SZ // N_CORES          # samples per core = 4
T = SPB * S                    # tokens per core = 2048
KT = D_IN // P                 # 32 k-tiles
T_BLOCK = 1024                 # tokens per resident block
N_TB = T // T_BLOCK            # 2 blocks
SL_PER_TB = T_BLOCK // S       # samples per block = 2
TT_PER_TB = T_BLOCK // P       # 8 t-tiles per block
O_TILE = 512
N_OT = D_OUT // O_TILE         # 8 o-tiles
TT_PER_S = S // P              # 4 t-tiles per sample
SCALING = 2.0

LAST_RESULTS = None            # test harness reads exec_time_ns from here

_COMPILED = {}


def _ensure_axon_hooks_module():
    """If the image's antenv lacks axon_hooks, install a no-op stub so
    run_bass_kernel_spmd(trace=...) degrades gracefully instead of
    raising ImportError."""
    try:
        import antenv.axon_hooks  # noqa: F401
        return
    except ImportError:
        pass
    try:
        import antenv
    except ImportError:
        return
    mod = types.ModuleType("antenv.axon_hooks")
    state = {"hook": None}
    mod.set_axon_ntff_profile_hook = lambda h: state.__setitem__("hook", h)
    mod.get_axon_ntff_profile_hook = lambda: state["hook"]
    sys.modules["antenv.axon_hooks"] = mod
    antenv.axon_hooks = mod


def _build():
    import concourse.bacc as bacc
    import concourse.bass as bass
    import concourse.mybir as mybir
    import concourse.tile as tile

    f32 = mybir.dt.float32
    f32r = mybir.dt.float32r

    nc = bacc.Bacc("TRN2", target_bir_lowering=False, debug=False,
                   enable_asserts=False)

    xt_d = nc.dram_tensor("xt", [P, KT, T], f32r, kind="ExternalInput").ap()
    wt_d = nc.dram_tensor("wt", [P, KT, D_OUT], f32r, kind="ExternalInput").ap()
    at_d = nc.dram_tensor("at", [P, SPB, KT, RANK], f32r, kind="ExternalInput").ap()
    bt_d = nc.dram_tensor("bt", [SPB, RANK, D_OUT], f32r, kind="ExternalInput").ap()
    b_d = nc.dram_tensor("bv", [D_OUT], f32, kind="ExternalInput").ap()
    out_d = nc.dram_tensor("out", [P, T // P, D_OUT], f32, kind="ExternalOutput").ap()

    with tile.TileContext(nc) as tc:
        with (
            tc.tile_pool(name="xt", bufs=KT + 1) as xt_pool,
            tc.tile_pool(name="wt", bufs=8) as wt_pool,
            tc.tile_pool(name="at", bufs=1) as at_pool,
            tc.tile_pool(name="bt", bufs=3) as bt_pool,
            tc.tile_pool(name="bias", bufs=2) as bias_pool,
            tc.tile_pool(name="inter", bufs=3) as inter_pool,
            tc.tile_pool(name="ob", bufs=8) as out_pool,
            tc.tile_pool(name="ps", bufs=8, space="PSUM") as ps_pool,
        ):
            # A^T resident, one tile per sample so each LoRA-1 chain
            # depends only on its own sample's load
            at_sbs = []
            for s in range(SPB):
                at_t = at_pool.tile([P, KT, RANK], f32r,
                                    name=f"at_{s}", tag=f"at_{s}")
                nc.scalar.dma_start(at_t[:], at_d[:, s])
                at_sbs.append(at_t)

            # k-direction per (tb, o): snakes so each consumer reads X
            # tiles in the order the previous phase frees them; block
            # arrival order then matches the next block's first reader.
            def k_order(asc):
                return list(range(KT)) if asc else list(range(KT - 1, -1, -1))

            def o_asc(tb, o):
                return (o % 2 == 0) if tb == 0 else (o % 2 == 1)

            N_FUSED = TT_PER_TB - SL_PER_TB   # t-tiles fused with LoRA-1

            def emit_bt_bias(tb, o):
                bts = []
                for sl in range(SL_PER_TB):
                    s = tb * SL_PER_TB + sl
                    bt_t = bt_pool.tile([RANK, O_TILE], f32r,
                                        name=f"bt_{tb}_{o}_{sl}", tag="bt")
                    nc.gpsimd.dma_start(
                        bt_t[:], bt_d[s, :, o * O_TILE:(o + 1) * O_TILE])
                    bts.append(bt_t)
                bias_t = bias_pool.tile([P, O_TILE], f32,
                                        name=f"bias_{tb}_{o}", tag="bias")
                bias_bcast = bass.AP(
                    tensor=b_d.tensor,
                    offset=o * O_TILE,
                    ap=[[0, P], [1, O_TILE]])
                nc.gpsimd.dma_start(out=bias_t[:], in_=bias_bcast)
                return bts, bias_t

            def emit_lora2(tb, o, tt, psums, inters, bts):
                sl = tt // TT_PER_S
                nc.tensor.matmul(
                    psums[tt][:],
                    inters[sl][:, (tt % TT_PER_S) * P:
                               (tt % TT_PER_S + 1) * P],
                    bts[sl][:],
                    start=False, stop=True)

            def emit_evict(tb, o, tt_list, psums, bias_t):
                for tt in tt_list:
                    o_t = out_pool.tile([P, O_TILE], f32,
                                        name=f"o_{tb}_{o}_{tt}", tag="o")
                    nc.vector.tensor_add(o_t[:], psums[tt][:], bias_t[:])
                    nc.scalar.dma_start(
                        out_d[:, tb * TT_PER_TB + tt,
                              o * O_TILE:(o + 1) * O_TILE],
                        o_t[:])

            for tb in range(N_TB):
                # ---- load this block's X^T k-tiles (progressively) ----
                # tb=0: nothing else queued yet, so split the DMA issue
                # across the SWDGE and ACT queues to halve the fill time
                kt_load_order = k_order(o_asc(tb, 0))
                xts = [None] * KT
                for i, kt in enumerate(kt_load_order):
                    xt_t = xt_pool.tile([P, T_BLOCK], f32r,
                                        name=f"xt_{tb}_{kt}", tag="xt")
                    nc.gpsimd.dma_start(
                        xt_t[:], xt_d[:, kt, tb * T_BLOCK:(tb + 1) * T_BLOCK])
                    xts[kt] = xt_t

                # ---- LoRA-1 prefill: inter^T[s], consuming X tiles in
                # arrival order ----
                inters = []
                for sl in range(SL_PER_TB):
                    s = tb * SL_PER_TB + sl
                    ps_i = ps_pool.tile([RANK, S], mybir.dt.float32,
                                        name=f"psi_{tb}_{sl}", tag="ps")
                    for j, kt in enumerate(kt_load_order):
                        nc.tensor.matmul(
                            ps_i[:],
                            at_sbs[s][:, kt, :],
                            xts[kt][:, sl * S:(sl + 1) * S],
                            start=(j == 0), stop=(j == KT - 1))
                    it_t = inter_pool.tile([RANK, S], f32r,
                                           name=f"it_{tb}_{sl}", tag="it")
                    nc.vector.tensor_copy(it_t[:], ps_i[:].bitcast(f32r))
                    inters.append(it_t)

                # ---- base matmul + LoRA-2 + bias, per o-tile ----
                for o in range(N_OT):
                    psums = [
                        ps_pool.tile([P, O_TILE], mybir.dt.float32,
                                     name=f"ps_{tb}_{o}_{i}", tag="ps")
                        for i in range(TT_PER_TB)
                    ]
                    bts, bias_t = emit_bt_bias(tb, o)
                    kts = k_order(o_asc(tb, o))
                    # skew: tt 0..3 run one k-row ahead of tt 4..7 so the
                    # 8 PSUM banks are demanded (and the previous o-tile's
                    # evictions consumed) staggered instead of all at once
                    HALF = TT_PER_TB // 2
                    w_tiles = []
                    for j, kt in enumerate(kts):
                        w_t = wt_pool.tile([P, O_TILE], f32r,
                                           name=f"w_{tb}_{o}_{kt}", tag="w")
                        nc.sync.dma_start(
                            w_t[:], wt_d[:, kt, o * O_TILE:(o + 1) * O_TILE])
                        w_tiles.append(w_t)
                        for tt in range(HALF):
                            nc.tensor.matmul(
                                psums[tt][:],
                                xts[kt][:, tt * P:(tt + 1) * P],
                                w_t[:],
                                start=(j == 0), stop=False)
                        if j >= 1:
                            ktb = kts[j - 1]
                            for tt in range(HALF, TT_PER_TB):
                                nc.tensor.matmul(
                                    psums[tt][:],
                                    xts[ktb][:, tt * P:(tt + 1) * P],
                                    w_tiles[j - 1][:],
                                    start=(j == 1), stop=False)
                    for tt in range(HALF):
                        emit_lora2(tb, o, tt, psums, inters, bts)
                    ktb = kts[KT - 1]
                    for tt in range(HALF, TT_PER_TB):
                        nc.tensor.matmul(
                            psums[tt][:],
                            xts[ktb][:, tt * P:(tt + 1) * P],
                            w_tiles[KT - 1][:],
                            start=False, stop=False)
                    emit_evict(tb, o, list(range(HALF)), psums, bias_t)
                    for tt in range(HALF, TT_PER_TB):
                        emit_lora2(tb, o, tt, psums, inters, bts)
                    emit_evict(tb, o, list(range(HALF, TT_PER_TB)), psums,
                               bias_t)

    nc.compile()
    return nc


def _get_compiled():
    if "nc" not in _COMPILED:
        _COMPILED["nc"] = _build()
    return _COMPILED["nc"]


def kernel(x, adapter_ids, A_all, B_all, W, b):
    global LAST_RESULTS
    _ensure_axon_hooks_module()
    from concourse.bass_utils import run_bass_kernel_spmd

    x = np.asarray(x, dtype=np.float32)
    adapter_ids = np.asarray(adapter_ids)
    A_all = np.asarray(A_all, dtype=np.float32)
    B_all = np.asarray(B_all, dtype=np.float32)
    W = np.asarray(W, dtype=np.float32)
    b = np.asarray(b, dtype=np.float32)

    nc = _get_compiled()

    # ---- host-side layout prep ----
    # W^T: wt[p, kt, o] = W[o, kt*128+p]
    wt_np = np.ascontiguousarray(
        W.T.reshape(KT, P, D_OUT).transpose(1, 0, 2))

    A_batch = A_all[adapter_ids]              # (B, R, D_IN)
    B_batch = B_all[adapter_ids] * SCALING    # (B, D_OUT, R) — exact *2 fold

    in_maps = []
    for c in range(N_CORES):
        # Rotate each core's view of the out-feature axis by c o-tiles:
        # the SPMD cores run in near-lockstep, and without the stagger
        # they all stream the same W bytes at the same instant, piling
        # onto the same HBM channels.
        sh = (c % N_OT) * O_TILE
        xs = x[c * SPB:(c + 1) * SPB].reshape(T, D_IN)
        xt_np = np.ascontiguousarray(
            xs.reshape(T, KT, P).transpose(2, 1, 0))            # [P, KT, T]
        A_c = A_batch[c * SPB:(c + 1) * SPB]                    # (SPB, R, D_IN)
        at_np = np.ascontiguousarray(
            A_c.reshape(SPB, RANK, KT, P).transpose(3, 0, 2, 1))  # [P,SPB,KT,R]
        B_c = B_batch[c * SPB:(c + 1) * SPB]                    # (SPB, D_OUT, R)
        bt_np = np.ascontiguousarray(
            np.roll(B_c.transpose(0, 2, 1), -sh, axis=2))       # [SPB, R, D_OUT]
        in_maps.append({
            "xt": xt_np, "wt": np.roll(wt_np, -sh, axis=2),
            "at": at_np, "bt": bt_np, "bv": np.roll(b, -sh),
        })

    res = run_bass_kernel_spmd(nc, in_maps, core_ids=list(range(N_CORES)))
    LAST_RESULTS = res

    out = np.empty((B_SZ, S, D_OUT), dtype=np.float32)
    for c in range(N_CORES):
        sh = (c % N_OT) * O_TILE
        oc = np.roll(res.results[c]["out"], sh, axis=2)         # [P, T//P, D_OUT]
        out[c * SPB:(c + 1) * SPB] = (
            oc.transpose(1, 0, 2).reshape(T, D_OUT).reshape(SPB, S, D_OUT))
    return out



# revision 4
# speedup vs baseline: 1.1300x; 1.1300x over previous
"""Batched LoRA Linear on 8 Trainium2 NeuronCores (Bass/Tile).

Computes, for x (32, 512, 4096), adapter_ids (32,), A_all (32, 16, 4096),
B_all (32, 4096, 16), W (4096, 4096), b (4096,):

    out = x @ W.T + b + 2.0 * ((x @ A[aid].T) @ B[aid].T)

Sharding: data-parallel over batch - 4 samples per core; W/b replicated.

Per-core device kernel (bf16 operands, fp32 PSUM accumulation):
  - x is converted to bf16 and kept FULLY resident in SBUF
    ([128, 32 k-tiles, 2048 tokens] = 128 KiB/partition), so W is
    streamed exactly once and there is a single phase (no t-blocks).
  - W-stationary matmuls: for each 128-wide output chunk (oc) and each
    k-tile, ONE weight load feeds 4 matmuls (the 4 x 512-token chunks),
    cutting LDWEIGHTS pressure 4x vs an x-stationary schedule; bf16
    weights additionally get fast-weight-load.
  - PSUM layout is [o_part=128, t=512]; 4 banks accumulate one oc while
    the previous oc's 4 banks drain (bias-add on DVE, then DMA out).
  - LoRA-1 (inter = A x^T per sample) runs packed in the 4 PE column
    groups (out partitions 32s..32s+15) during the x fill, fused into
    oc0's k-loop so the tensor engine never idles while x streams in.
  - LoRA-2 is one K=16 matmul per (oc, sample) accumulated into the
    same PSUM bank as the base matmul before eviction; the 4 samples
    occupy the 4 PE row groups (B rows at partitions 32s..32s+15) so
    they can execute concurrently.  oc0's LoRA-2 is applied late (after
    inter is ready) via separate PSUM + a DVE add.

Host side only reshapes/transposes/gathers/dtype-converts (no
arithmetic except the exact *2.0 fold into B).
"""

import sys
import types

import numpy as np

# ---------------------------------------------------------------- constants
P = 128
B_SZ = 32            # batch
S = 512              # seq len
D_IN = 4096
D_OUT = 4096
RANK = 16
N_CORES = 8
SPB = B_SZ // N_CORES          # samples per core = 4
T = SPB * S                    # tokens per core = 2048
KT = D_IN // P                 # 32 k-tiles
OC = D_OUT // P                # 32 output chunks of 128
TC = T // S                    # 4 token chunks of 512 (chunk == sample)
SCALING = 2.0

LAST_RESULTS = None            # test harness reads exec_time_ns from here

_COMPILED = {}


def _ensure_axon_hooks_module():
    """If the image's antenv lacks axon_hooks, install a no-op stub so
    run_bass_kernel_spmd(trace=...) degrades gracefully instead of
    raising ImportError."""
    try:
        import antenv.axon_hooks  # noqa: F401
        return
    except ImportError:
        pass
    try:
        import antenv
    except ImportError:
        return
    mod = types.ModuleType("antenv.axon_hooks")
    state = {"hook": None}
    mod.set_axon_ntff_profile_hook = lambda h: state.__setitem__("hook", h)
    mod.get_axon_ntff_profile_hook = lambda: state["hook"]
    sys.modules["antenv.axon_hooks"] = mod
    antenv.axon_hooks = mod


def _build():
    import concourse.bacc as bacc
    import concourse.bass as bass  # noqa: F401
    import concourse.mybir as mybir
    import concourse.tile as tile

    f32 = mybir.dt.float32
    bf16 = mybir.dt.bfloat16

    nc = bacc.Bacc("TRN2", target_bir_lowering=False, debug=False,
                   enable_asserts=False)

    xt_d = nc.dram_tensor("xt", [P, KT, T], bf16, kind="ExternalInput").ap()
    wt_d = nc.dram_tensor("wt", [P, OC, KT, P], bf16, kind="ExternalInput").ap()
    at_d = nc.dram_tensor("at", [P, SPB, KT, RANK], bf16,
                          kind="ExternalInput").ap()
    bt_d = nc.dram_tensor("bt", [P, D_OUT], bf16, kind="ExternalInput").ap()
    bc_d = nc.dram_tensor("bc", [P, OC], f32, kind="ExternalInput").ap()
    out_d = nc.dram_tensor("out", [P, OC, T], f32, kind="ExternalOutput").ap()

    with tile.TileContext(nc) as tc:
        with (
            tc.tile_pool(name="xt", bufs=KT) as xt_pool,
            tc.tile_pool(name="wt", bufs=3) as wt_pool,
            tc.tile_pool(name="misc", bufs=1) as misc_pool,
            tc.tile_pool(name="ob", bufs=8) as out_pool,
            tc.tile_pool(name="ps", bufs=8, space="PSUM") as ps_pool,
        ):
            # ---- prologue DMAs ----
            # sync queue order: at (needed by the first MMs), wt slab 0,
            # then bt/bc (needed only ~45us in).
            at_sb = misc_pool.tile([P, SPB, KT, RANK], bf16,
                                   name="at_sb", tag="at")
            nc.sync.dma_start(at_sb[:], at_d[:])
            wt0 = wt_pool.tile([P, KT, P], bf16, name="wt_0", tag="wt")
            nc.sync.dma_start(wt0[:], wt_d[:, 0])
            bt_sb = misc_pool.tile([P, D_OUT], bf16, name="bt_sb", tag="bt")
            nc.sync.dma_start(bt_sb[:], bt_d[:])
            bc_sb = misc_pool.tile([P, OC], f32, name="bc_sb", tag="bc")
            nc.sync.dma_start(bc_sb[:], bc_d[:])
            inter_sb = misc_pool.tile([P, S], bf16, name="inter_sb",
                                      tag="inter")

            # x: full-residency load, split across the gpsimd and scalar
            # DMA queues so the fill approaches the per-core HBM share.
            xqueues = [nc.gpsimd, nc.scalar]
            xts = []
            for kt in range(KT):
                xt_t = xt_pool.tile([P, T], bf16, name=f"xt_{kt}", tag="xt")
                xqueues[kt % 2].dma_start(xt_t[:], xt_d[:, kt])
                xts.append(xt_t)

            # ---- oc0 base + LoRA-1, fused per k-tile ----
            # lora1 for sample s lands at psum partitions 32s..32s+15
            # (PE column group s) so the 4 samples run concurrently.
            ps_l = [ps_pool.tile([P, S], f32, name=f"psl_{s}", tag="ps")
                    for s in range(SPB)]
            ps0 = [ps_pool.tile([P, S], f32, name=f"ps0_{t}", tag="ps")
                   for t in range(TC)]
            for kt in range(KT):
                for s in range(SPB):
                    nc.tensor.matmul(
                        ps_l[s][32 * s:32 * s + RANK, :],
                        at_sb[:, s, kt, :],
                        xts[kt][:, s * S:(s + 1) * S],
                        start=(kt == 0), stop=(kt == KT - 1),
                        tile_position=(0, 32 * s))
                for t in range(TC):
                    nc.tensor.matmul(
                        ps0[t][:, :],
                        wt0[:, kt, :],
                        xts[kt][:, t * S:(t + 1) * S],
                        start=(kt == 0), stop=(kt == KT - 1))

            # inter (bf16) at partitions 32s..32s+15; frees lora1 banks
            for s in range(SPB):
                nc.vector.tensor_copy(inter_sb[32 * s:32 * s + RANK, :],
                                      ps_l[s][32 * s:32 * s + RANK, :])
            # oc0 base-only eviction (frees ps0 banks); held until the
            # late lora2 add below
            o0_tiles = []
            for t in range(TC):
                o0_t = out_pool.tile([P, S], f32, name=f"o0_{t}", tag="o0",
                                     bufs=SPB)
                nc.vector.tensor_scalar_add(o0_t[:], ps0[t][:], bc_sb[:, 0:1])
                o0_tiles.append(o0_t)
            # late lora2 for oc0 (inter is ready now)
            for t in range(TC):
                s = t
                pl2 = ps_pool.tile([P, S], f32, name=f"pl2_{t}", tag="ps")
                nc.tensor.matmul(
                    pl2[:, :],
                    bt_sb[32 * s:32 * s + RANK, 0:P],
                    inter_sb[32 * s:32 * s + RANK, :],
                    start=True, stop=True, tile_position=(32 * s, 0))
                f_t = out_pool.tile([P, S], f32, name=f"of0_{t}", tag="o")
                nc.vector.tensor_add(f_t[:], o0_tiles[t][:], pl2[:])
                nc.scalar.dma_start(out_d[:, 0, t * S:(t + 1) * S], f_t[:])

            # ---- oc 1..31: base k-loop + fused lora2 + eviction ----
            for oc in range(1, OC):
                wt_t = wt_pool.tile([P, KT, P], bf16, name=f"wt_{oc}",
                                    tag="wt")
                nc.sync.dma_start(wt_t[:], wt_d[:, oc])
                psums = [ps_pool.tile([P, S], f32, name=f"ps_{oc}_{t}",
                                      tag="ps")
                         for t in range(TC)]
                for kt in range(KT):
                    for t in range(TC):
                        nc.tensor.matmul(
                            psums[t][:, :],
                            wt_t[:, kt, :],
                            xts[kt][:, t * S:(t + 1) * S],
                            start=(kt == 0), stop=False)
                for t in range(TC):
                    s = t
                    nc.tensor.matmul(
                        psums[t][:, :],
                        bt_sb[32 * s:32 * s + RANK, oc * P:(oc + 1) * P],
                        inter_sb[32 * s:32 * s + RANK, :],
                        start=False, stop=True, tile_position=(32 * s, 0))
                for t in range(TC):
                    o_t = out_pool.tile([P, S], f32, name=f"o_{oc}_{t}",
                                        tag="o")
                    nc.vector.tensor_scalar_add(o_t[:], psums[t][:],
                                                bc_sb[:, oc:oc + 1])
                    nc.scalar.dma_start(out_d[:, oc, t * S:(t + 1) * S],
                                        o_t[:])

    nc.compile()
    return nc


def _get_compiled():
    if "nc" not in _COMPILED:
        _COMPILED["nc"] = _build()
    return _COMPILED["nc"]


def kernel(x, adapter_ids, A_all, B_all, W, b):
    global LAST_RESULTS
    _ensure_axon_hooks_module()
    import ml_dtypes
    from concourse.bass_utils import run_bass_kernel_spmd

    bf16 = ml_dtypes.bfloat16

    x = np.asarray(x, dtype=np.float32)
    adapter_ids = np.asarray(adapter_ids)
    A_all = np.asarray(A_all, dtype=np.float32)
    B_all = np.asarray(B_all, dtype=np.float32)
    W = np.asarray(W, dtype=np.float32)
    b = np.asarray(b, dtype=np.float32)

    nc = _get_compiled()

    # ---- host-side layout prep (reshape/transpose/gather/dtype only) ----
    # wt[p, oc, kt, o'] = W[oc*128+o', kt*128+p]
    wt_np = np.ascontiguousarray(
        W.astype(bf16).reshape(OC, P, KT, P).transpose(3, 0, 2, 1))
    # bc[p, oc] = b[oc*128+p]
    bc_np = np.ascontiguousarray(b.reshape(OC, P).T)

    A_batch = A_all[adapter_ids]              # (B, R, D_IN)
    B_batch = B_all[adapter_ids] * SCALING    # (B, D_OUT, R) - exact *2 fold

    in_maps = []
    for c in range(N_CORES):
        xs = x[c * SPB:(c + 1) * SPB].reshape(T, D_IN).astype(bf16)
        # xt[p, kt, t] = x_core[t, kt*128+p]
        xt_np = np.ascontiguousarray(
            xs.reshape(T, KT, P).transpose(2, 1, 0))
        A_c = A_batch[c * SPB:(c + 1) * SPB].astype(bf16)   # (SPB, R, D_IN)
        # at[p, s, kt, r] = A_c[s, r, kt*128+p]
        at_np = np.ascontiguousarray(
            A_c.reshape(SPB, RANK, KT, P).transpose(3, 0, 2, 1))
        B_c = B_batch[c * SPB:(c + 1) * SPB].astype(bf16)   # (SPB, D_OUT, R)
        # bt[32s+r, o] = 2*B_c[s][o, r]
        bt_np = np.zeros((P, D_OUT), dtype=bf16)
        for s in range(SPB):
            bt_np[32 * s:32 * s + RANK, :] = B_c[s].T
        in_maps.append({
            "xt": xt_np, "wt": wt_np, "at": at_np, "bt": bt_np,
            "bc": bc_np,
        })

    res = run_bass_kernel_spmd(nc, in_maps, core_ids=list(range(N_CORES)))
    LAST_RESULTS = res

    out = np.empty((B_SZ, S, D_OUT), dtype=np.float32)
    for c in range(N_CORES):
        oc_np = res.results[c]["out"]              # [p, oc, t]
        out[c * SPB:(c + 1) * SPB] = (
            oc_np.transpose(2, 1, 0).reshape(T, D_OUT)
            .reshape(SPB, S, D_OUT))
    return out
